# revision 10
# baseline (speedup 1.0000x reference)
"""Trainium2 Bass kernel for nn_DenseSparsePreEmbedding.

Math refactoring (bit-exact vs the jax reference in f32):
    fixed_emb @ W_fixed  == (fixed_table @ W_fixed)[fixed_features]
    sparse_emb @ W_sparse== (concat(tabs) @ W_sparse)[cv]  with cv the
                            combined per-token sparse code (last write wins,
                            sentinel 256 -> zero row for untouched tokens)
so the whole module collapses to a dual embedding gather + add:
    out[n] = tabA[ffn] + tabB[cvn]
with tabA = fixed_table @ W_fixed + b   [2048, 128]
     tabB = concat(tab0..3) @ W_sparse (+ zero row)  [257, 128]

Perf learnings (HW traces, 8-core trn2):
  - The 16 SDMA engines are the binding resource (~93% busy). Cost is per
    packet (~25ns/512B), so halving row bytes (fp16) and fattening store
    packets both pay directly. MBU is <10% — HBM bandwidth itself is idle.
  - The raw 128KB tabB was the killer: 61% of its gathers hit one sentinel
    row -> DRAM bank serialization at 4.4GB/s/engine vs 20GB/s for tabA.
    Fix: replicate tabB KREP times in HBM and cycle replicas per stream
    slot; B-gathers then run at A-gather efficiency.
  - fp16 keeps rel err ~1e-4 (values ~1e-2); tolerance is 2e-2.
  - Global token permutation: partition p owns output rows
    [p*RPC, (p+1)*RPC) so each per-tile store is fb contiguous rows
    (fb*256B) per partition instead of 4 scattered 512B rows.
Device kernel per 512-token tile: two gpsimd.dma_gather (256B rows) on
4 SWDGE queues, DVE fp16 add, HWDGE store.
"""

import os as _os

import numpy as np

N = 1_000_000
NCORES = 8
PER = N // NCORES          # 125000 tokens per core
V = 2048
D = 128
NSPARSE = 257              # 4*64 sparse rows + zero sentinel row

TT = int(_os.environ.get("KTT", "512"))     # tokens per tile (ring limit: <=1920 idx/op)
NQUEUES = int(_os.environ.get("KNQ", "4"))  # SWDGE queues to spread gathers over
BUFS = int(_os.environ.get("KBUFS", "8"))   # work tile-pool buffers
CHUNK = int(_os.environ.get("KCHUNK", "8")) # split idx preloads into N chunks
REP = int(_os.environ.get("KREP", "64"))    # tabB HBM replicas (257*REP <= 32767)
RA = int(_os.environ.get("KRA", "1"))       # tabA HBM replicas (2048*RA <= 32767)
DTS = _os.environ.get("KDT", "f16")         # f16 | bf16 | f32
SSPLIT = int(_os.environ.get("KSSPLIT", "0"))  # alternate stores sync/scalar HWDGE
GST = int(_os.environ.get("KGST", "1"))     # tiles per superstore (1 = store each tile)

PEB = int(_os.environ.get("KPEB", "0"))     # B-table via PE one-hot matmul
SCRATCH = int(_os.environ.get("KSCRATCH", "0"))  # dynamic_dma_scratch_size (0=default)
OHTT = int(_os.environ.get("KOHTT", "1"))   # one-hot via tensor_tensor broadcast
BCMM = int(_os.environ.get("KBCMM", "0"))   # cv broadcast via K=1 ones-matmul
FB = TT // 128             # output rows per partition per full tile
RPC = -(-PER // 128)       # 977 output rows owned by each partition
PADTOK = RPC * 128         # 125056 padded per-core tokens

_cache = {}


def _npdt():
    import ml_dtypes

    return {"f16": np.float16, "bf16": ml_dtypes.bfloat16, "f32": np.float32}[DTS]


def _build_nc():
    import concourse.bacc as bacc
    import concourse.mybir as mybir
    import concourse.tile as tile

    dt = {
        "f16": mybir.dt.float16,
        "bf16": mybir.dt.bfloat16,
        "f32": mybir.dt.float32,
    }[DTS]
    nfull = PER // TT                    # 244 full tiles
    tail_rows = RPC - nfull * FB         # 1 leftover row per partition
    ntiles = nfull + (1 if tail_rows else 0)
    cols = PADTOK // 16

    kw = {}
    if NQUEUES > 1:
        kw["num_swdge_queues"] = NQUEUES
    nc = bacc.Bacc(
        "TRN2",
        target_bir_lowering=False,
        debug=False,
        enable_asserts=False,
        **kw,
    )
    idxa_t = nc.dram_tensor("idxa", [128, cols], mybir.dt.int16, kind="ExternalInput")
    idxb_t = nc.dram_tensor("idxb", [128, cols], mybir.dt.int16, kind="ExternalInput")
    taba_t = nc.dram_tensor("taba", [V * RA, D], dt, kind="ExternalInput")
    tabb_t = nc.dram_tensor("tabb", [NSPARSE * REP, D], dt, kind="ExternalInput")
    out_t = nc.dram_tensor("out", [PADTOK, D], dt, kind="ExternalOutput")

    idxa = idxa_t.ap()
    idxb = idxb_t.ap()
    taba = taba_t.ap()
    tabb = tabb_t.ap()
    out = out_t.ap()
    outv = out.rearrange("(p r) e -> p r e", p=128)

    with tile.TileContext(nc) as tc:
        with (
            tc.tile_pool(name="idxp", bufs=1) as ip,
            tc.tile_pool(name="work", bufs=BUFS) as wp,
        ):
            ia = ip.tile([128, cols], mybir.dt.int16, tag="ia")
            ib = ip.tile([128, cols], mybir.dt.int16, tag="ib")
            if CHUNK > 1:
                # chunked preload: first gathers only wait on their own chunk
                step = (cols + CHUNK - 1) // CHUNK
                for c0_ in range(0, cols, step):
                    c1_ = min(c0_ + step, cols)
                    nc.sync.dma_start(out=ia[:, c0_:c1_], in_=idxa[:, c0_:c1_])
                    nc.sync.dma_start(out=ib[:, c0_:c1_], in_=idxb[:, c0_:c1_])
            else:
                nc.sync.dma_start(out=ia[:], in_=idxa)
                nc.sync.dma_start(out=ib[:], in_=idxb)

            for t in range(ntiles):
                fb = FB if t < nfull else tail_rows
                ni = fb * 128
                c0 = (t * TT) // 16
                da = wp.tile([128, fb, 128], dt, tag="da")
                db = wp.tile([128, fb, 128], dt, tag="db")
                if NQUEUES > 1:
                    qa = (2 * t) % NQUEUES
                    qb = (2 * t + 1) % NQUEUES
                else:
                    qa = qb = 0
                nc.gpsimd.dma_gather(
                    da[:], taba, ia[:, c0 : c0 + ni // 16], ni, ni, D,
                    queue_num=qa,
                )
                nc.gpsimd.dma_gather(
                    db[:], tabb, ib[:, c0 : c0 + ni // 16], ni, ni, D,
                    queue_num=qb,
                )
                nc.vector.tensor_add(out=da[:], in0=da[:], in1=db[:])
                st = nc.scalar if (SSPLIT and t % 2) else nc.sync
                r0 = t * FB
                st.dma_start(out=outv[:, r0 : r0 + fb, :], in_=da[:])
    nc.compile()
    return nc


def _build_nc_peb():
    """A-table via dma_gather; B-table on the TensorEngine.

    Per tile: DVE builds two code-on-partition one-hots (is_equal of the
    host-prebroadcast cv stream against per-partition iota columns), PE
    contracts them against the SBUF-resident 256-row B table into PSUM
    ([tok, e]), DVE adds A-gather + PSUM, HWDGE stores. Removes the B
    gathers from both the SWDGE queues (Q7 descriptor-gen was the
    bottleneck) and the SDMA random-read load."""
    import concourse.bacc as bacc
    import concourse.mybir as mybir
    import concourse.tile as tile

    assert DTS == "f16"
    dt = mybir.dt.float16
    nfull = PER // TT
    tail_rows = RPC - nfull * FB
    ntiles = nfull + (1 if tail_rows else 0)
    cols = PADTOK // 16

    kw = {}
    if NQUEUES > 1:
        kw["num_swdge_queues"] = NQUEUES
    if SCRATCH:
        kw["dynamic_dma_scratch_size"] = SCRATCH
    nc = bacc.Bacc(
        "TRN2",
        target_bir_lowering=False,
        debug=False,
        enable_asserts=False,
        **kw,
    )
    nfull_ = PER // TT
    ntiles_ = nfull_ + 1
    tpp = -(-ntiles_ // 3)             # tiles per cvrow base partition (0/32/64)
    idxa_t = nc.dram_tensor("idxa", [128, cols], mybir.dt.int16, kind="ExternalInput")
    taba_t = nc.dram_tensor("taba", [V * RA, D], dt, kind="ExternalInput")
    tabbs_t = nc.dram_tensor("tabbs", [256, D], dt, kind="ExternalInput")
    if BCMM:
        cvr_t = nc.dram_tensor("cvrow", [65, tpp * TT], dt, kind="ExternalInput")
        ones_t = nc.dram_tensor("ones1", [65, 128], dt, kind="ExternalInput")
    else:
        cvbc_t = nc.dram_tensor("cvbc", [128, PADTOK], dt, kind="ExternalInput")
    iota_t = nc.dram_tensor("iota2", [128, 2, TT], dt, kind="ExternalInput")
    out_t = nc.dram_tensor("out", [PADTOK, D], dt, kind="ExternalOutput")

    idxa = idxa_t.ap()
    taba = taba_t.ap()
    tabbs = tabbs_t.ap()
    if BCMM:
        cvr = cvr_t.ap()
        ones1 = ones_t.ap()
    else:
        cvbc = cvbc_t.ap()
    iota2 = iota_t.ap()
    out = out_t.ap()
    outv = out.rearrange("(p r) e -> p r e", p=128)

    with tile.TileContext(nc) as tc:
        with (
            tc.tile_pool(name="idxp", bufs=1) as ip,
            tc.tile_pool(name="work", bufs=BUFS) as wp,
            tc.tile_pool(name="psum", bufs=2 if BCMM else 4, space="PSUM") as pp,
            tc.tile_pool(name="psbc", bufs=2, space="PSUM") as pb,
        ):
            ia = ip.tile([128, cols], mybir.dt.int16, tag="ia")
            tbl = ip.tile([128, 2, 128], dt, tag="tb")   # [code, chunk, e]
            io2 = ip.tile([128, 2, TT], dt, tag="io")
            nc.sync.dma_start(out=tbl[:, 0, :], in_=tabbs[0:128, :])
            nc.sync.dma_start(out=tbl[:, 1, :], in_=tabbs[128:256, :])
            nc.sync.dma_start(out=io2[:], in_=iota2)
            if BCMM:
                cvrs = ip.tile([65, tpp * TT], dt, tag="cr")
                on1 = ip.tile([65, 128], dt, tag="on")
                nc.sync.dma_start(out=cvrs[:], in_=cvr)
                nc.sync.dma_start(out=on1[:], in_=ones1)
            if CHUNK > 1:
                step = (cols + CHUNK - 1) // CHUNK
                for c0_ in range(0, cols, step):
                    c1_ = min(c0_ + step, cols)
                    nc.sync.dma_start(out=ia[:, c0_:c1_], in_=idxa[:, c0_:c1_])
            else:
                nc.sync.dma_start(out=ia[:], in_=idxa)

            for t in range(ntiles):
                fb = FB if t < nfull else tail_rows
                ni = fb * 128
                c0 = (t * TT) // 16
                da = wp.tile([128, fb, 128], dt, tag="da")
                cvt = wp.tile([128, fb * 128], dt, tag="cv")
                oh = wp.tile([128, 2, fb, 128], dt, tag="oh")
                ps = pp.tile([128, fb * 128], mybir.dt.float32, tag="ps")
                psv = ps[:].rearrange("p (b e) -> p b e", e=128)
                qa = t % NQUEUES if NQUEUES > 1 else 0
                nc.gpsimd.dma_gather(
                    da[:], taba, ia[:, c0 : c0 + ni // 16], ni, ni, D,
                    queue_num=qa,
                )
                if BCMM:
                    pbc = pb.tile([128, fb * 128], mybir.dt.float32, tag="pb")
                    base = 32 * (t // tpp)             # 0, 32, or 64
                    j_ = t % tpp
                    for h0 in range(0, ni, 512):
                        h1 = min(h0 + 512, ni)
                        nc.tensor.matmul(
                            pbc[:, h0:h1],
                            on1[base : base + 1, :],
                            cvrs[base : base + 1, j_ * TT + h0 : j_ * TT + h1],
                            start=True,
                            stop=True,
                        )
                    nc.scalar.activation(
                        out=cvt[:],
                        in_=pbc[:],
                        func=mybir.ActivationFunctionType.Copy,
                    )
                else:
                    nc.scalar.dma_start(
                        out=cvt[:], in_=cvbc[:, t * TT : t * TT + ni]
                    )
                for ch in range(2):
                    ohv = oh[:, ch, :, :].rearrange("p b e -> p (b e)")
                    nc.vector.tensor_tensor(
                        out=ohv,
                        in0=cvt[:],
                        in1=io2[:, ch, : fb * 128],
                        op=mybir.AluOpType.is_equal,
                    )
                for b in range(fb):
                    nc.tensor.matmul(
                        psv[:, b, :],
                        oh[:, 0, b, :],
                        tbl[:, 0, :],
                        start=True,
                        stop=False,
                    )
                    nc.tensor.matmul(
                        psv[:, b, :],
                        oh[:, 1, b, :],
                        tbl[:, 1, :],
                        start=False,
                        stop=True,
                    )
                bb = wp.tile([128, fb, 128], dt, tag="bb")
                nc.scalar.activation(
                    out=bb[:].rearrange("p b e -> p (b e)"),
                    in_=ps[:],
                    func=mybir.ActivationFunctionType.Copy,
                )
                nc.vector.tensor_add(out=da[:], in0=da[:], in1=bb[:])
                r0 = t * FB
                nc.sync.dma_start(out=outv[:, r0 : r0 + fb, :], in_=da[:])
    nc.compile()
    return nc


def _get_nc():
    if "nc" not in _cache:
        _cache["nc"] = _build_nc_peb() if PEB else _build_nc()
    return _cache["nc"]


def _wrap_idx(arr_i16):
    """[PADTOK] int16 -> [128, COLS] dma_gather wrapped layout: stream slot i
    lives at [i % 16, i // 16]; the 16-row block is replicated to fill 128
    partitions."""
    w16 = arr_i16.reshape(-1, 16).T  # [16, COLS]
    return np.ascontiguousarray(np.tile(w16, (8, 1)))  # [128, COLS]


def _stream_token_order():
    """token id carried by each gather-stream slot.

    Stream slot s = t*TT + b*128 + p (tile t, block b, partition p) maps to
    output row p*RPC + t*FB + b, so each partition's store per tile is FB
    contiguous rows. The tail tile has tail_rows blocks."""
    nfull = PER // TT
    tail_rows = RPC - nfull * FB
    t_ = np.arange(nfull)[:, None, None]
    b_ = np.arange(FB)[None, :, None]
    p_ = np.arange(128)[None, None, :]
    head = (p_ * RPC + t_ * FB + b_).reshape(-1)
    parts = [head]
    if tail_rows:
        tb_ = np.arange(tail_rows)[:, None]
        tp_ = np.arange(128)[None, :]
        parts.append((tp_ * RPC + nfull * FB + tb_).reshape(-1))
    return np.concatenate(parts)


def kernel(
    fixed_features,
    idx0, val0, idx1, val1, idx2, val2, idx3, val3,
    fixed_table, tab0, tab1, tab2, tab3, W_fixed, W_sparse, b,
):
    from concourse.bass_utils import run_bass_kernel_spmd

    npdt = _npdt()
    ff = np.asarray(fixed_features)
    # combined sparse code per token; 256 = untouched sentinel (zero row).
    cv = np.full(N, 256, dtype=np.int32)
    for k, (ii, vv) in enumerate(
        ((idx0, val0), (idx1, val1), (idx2, val2), (idx3, val3))
    ):
        cv[np.asarray(ii)] = k * 64 + np.asarray(vv).astype(np.int32)

    ft = np.asarray(fixed_table, dtype=np.float32)
    wf = np.asarray(W_fixed, dtype=np.float32)
    ws = np.asarray(W_sparse, dtype=np.float32)
    bb = np.asarray(b, dtype=np.float32)
    taba1 = (ft @ wf + bb).astype(npdt)
    taba = np.ascontiguousarray(np.tile(taba1, (RA, 1)))
    tabs = np.concatenate(
        [np.asarray(t, dtype=np.float32) for t in (tab0, tab1, tab2, tab3)], axis=0
    )
    tabb1 = np.concatenate([tabs @ ws, np.zeros((1, D), np.float32)], axis=0)
    tabb = np.ascontiguousarray(np.tile(tabb1.astype(npdt), (REP, 1)))

    tok = _stream_token_order()                      # [PADTOK]
    valid = tok < PER
    tokc = np.where(valid, tok, 0)
    sr = np.arange(PADTOK)
    boff = (sr % REP).astype(np.int32) * NSPARSE     # B replica per stream slot
    aoff = (sr % RA).astype(np.int32) * V if RA > 1 else 0
    if PEB:
        iocol = np.stack([np.arange(128), 128 + np.arange(128)], axis=1)
        iota2 = np.ascontiguousarray(
            np.broadcast_to(iocol[:, :, None], (128, 2, TT))
        ).astype(npdt)
        tabbs = np.ascontiguousarray(tabb1[:256].astype(npdt))

    in_maps = []
    for c in range(NCORES):
        sl = slice(c * PER, (c + 1) * PER)
        ffc = np.asarray(ff[sl]).astype(np.int32)[tokc]
        cvc = cv[sl][tokc]
        fa = np.where(valid, ffc + aoff, 0).astype(np.int16)
        if PEB:
            cvs = np.where(valid, cvc, 256).astype(npdt)
            m = {
                "idxa": _wrap_idx(fa),
                "taba": taba,
                "tabbs": tabbs,
                "iota2": iota2,
            }
            if BCMM:
                nfull_ = PER // TT
                ntiles_ = nfull_ + 1
                tpp = -(-ntiles_ // 3)
                cvrow = np.full((65, tpp * TT), 256, dtype=npdt)
                for t_ in range(ntiles_):
                    seg = cvs[t_ * TT : min((t_ + 1) * TT, cvs.size)]
                    base, j_ = 32 * (t_ // tpp), t_ % tpp
                    cvrow[base, j_ * TT : j_ * TT + seg.size] = seg
                m["cvrow"] = np.ascontiguousarray(cvrow)
                ones65 = np.zeros((65, 128), dtype=npdt)
                ones65[[0, 32, 64], :] = 1
                m["ones1"] = ones65
            else:
                m["cvbc"] = np.ascontiguousarray(
                    np.broadcast_to(cvs[None, :], (128, PADTOK))
                )
            in_maps.append(m)
            continue
        fbv = np.where(valid, cvc + boff, 256).astype(np.int16)
        in_maps.append(
            {
                "idxa": _wrap_idx(fa),
                "idxb": _wrap_idx(fbv),
                "taba": taba,
                "tabb": tabb,
            }
        )

    nc = _get_nc()
    res = run_bass_kernel_spmd(nc, in_maps, core_ids=list(range(NCORES)))
    _cache["last_results"] = res
    out = np.concatenate(
        [
            np.asarray(res.results[c]["out"][:PER], dtype=np.float32)
            for c in range(NCORES)
        ],
        axis=0,
    )
    return out


# revision 11
# speedup vs baseline: 1.0761x; 1.0761x over previous
"""Trainium2 Bass kernel for nn_DenseSparsePreEmbedding.

Math refactoring (bit-exact vs the jax reference in f32):
    fixed_emb @ W_fixed  == (fixed_table @ W_fixed)[fixed_features]
    sparse_emb @ W_sparse== (concat(tabs) @ W_sparse)[cv]  with cv the
                            combined per-token sparse code (last write wins,
                            sentinel 256 -> zero row for untouched tokens)
so the whole module collapses to a dual embedding gather + add:
    out[n] = tabA[ffn] + tabB[cvn]
with tabA = fixed_table @ W_fixed + b   [2048, 128]
     tabB = concat(tab0..3) @ W_sparse (+ zero row)  [257, 128]

Perf learnings (HW traces, 8-core trn2):
  - The 16 SDMA engines are the binding resource (~93% busy). Cost is per
    packet (~25ns/512B), so halving row bytes (fp16) and fattening store
    packets both pay directly. MBU is <10% — HBM bandwidth itself is idle.
  - The raw 128KB tabB was the killer: 61% of its gathers hit one sentinel
    row -> DRAM bank serialization at 4.4GB/s/engine vs 20GB/s for tabA.
    Fix: replicate tabB KREP times in HBM and cycle replicas per stream
    slot; B-gathers then run at A-gather efficiency.
  - fp16 keeps rel err ~1e-4 (values ~1e-2); tolerance is 2e-2.
  - Global token permutation: partition p owns output rows
    [p*RPC, (p+1)*RPC) so each per-tile store is fb contiguous rows
    (fb*256B) per partition instead of 4 scattered 512B rows.
Device kernel per 512-token tile: two gpsimd.dma_gather (256B rows) on
4 SWDGE queues, DVE fp16 add, HWDGE store.
"""

import os as _os

import numpy as np

N = 1_000_000
NCORES = 8
PER = N // NCORES          # 125000 tokens per core
V = 2048
D = 128
NSPARSE = 257              # 4*64 sparse rows + zero sentinel row

TT = int(_os.environ.get("KTT", "512"))     # tokens per tile (ring limit: <=1920 idx/op)
NQUEUES = int(_os.environ.get("KNQ", "4"))  # SWDGE queues to spread gathers over
BUFS = int(_os.environ.get("KBUFS", "8"))   # work tile-pool buffers
CHUNK = int(_os.environ.get("KCHUNK", "8")) # split idx preloads into N chunks
REP = int(_os.environ.get("KREP", "64"))    # tabB HBM replicas (257*REP <= 32767)
RA = int(_os.environ.get("KRA", "1"))       # tabA HBM replicas (2048*RA <= 32767)
DTS = _os.environ.get("KDT", "f16")         # f16 | bf16 | f32
SSPLIT = int(_os.environ.get("KSSPLIT", "0"))  # alternate stores sync/scalar HWDGE
GST = int(_os.environ.get("KGST", "1"))     # tiles per superstore (1 = store each tile)

PEB = int(_os.environ.get("KPEB", "0"))     # B-table via PE one-hot matmul
SCRATCH = int(_os.environ.get("KSCRATCH", "0"))  # dynamic_dma_scratch_size (0=default)
OHTT = int(_os.environ.get("KOHTT", "1"))   # one-hot via tensor_tensor broadcast
BCMM = int(_os.environ.get("KBCMM", "0"))   # cv broadcast via K=1 ones-matmul
U8 = int(_os.environ.get("KU8", "0"))       # uint8 cv stream + extended tabA
SPLITG = int(_os.environ.get("KSPLITG", "0"))  # two half-gathers per tile
FB = TT // 128             # output rows per partition per full tile
RPC = -(-PER // 128)       # 977 output rows owned by each partition
PADTOK = RPC * 128         # 125056 padded per-core tokens

_cache = {}


def _npdt():
    import ml_dtypes

    return {"f16": np.float16, "bf16": ml_dtypes.bfloat16, "f32": np.float32}[DTS]


def _build_nc():
    import concourse.bacc as bacc
    import concourse.mybir as mybir
    import concourse.tile as tile

    dt = {
        "f16": mybir.dt.float16,
        "bf16": mybir.dt.bfloat16,
        "f32": mybir.dt.float32,
    }[DTS]
    nfull = PER // TT                    # 244 full tiles
    tail_rows = RPC - nfull * FB         # 1 leftover row per partition
    ntiles = nfull + (1 if tail_rows else 0)
    cols = PADTOK // 16

    kw = {}
    if NQUEUES > 1:
        kw["num_swdge_queues"] = NQUEUES
    nc = bacc.Bacc(
        "TRN2",
        target_bir_lowering=False,
        debug=False,
        enable_asserts=False,
        **kw,
    )
    idxa_t = nc.dram_tensor("idxa", [128, cols], mybir.dt.int16, kind="ExternalInput")
    idxb_t = nc.dram_tensor("idxb", [128, cols], mybir.dt.int16, kind="ExternalInput")
    taba_t = nc.dram_tensor("taba", [V * RA, D], dt, kind="ExternalInput")
    tabb_t = nc.dram_tensor("tabb", [NSPARSE * REP, D], dt, kind="ExternalInput")
    out_t = nc.dram_tensor("out", [PADTOK, D], dt, kind="ExternalOutput")

    idxa = idxa_t.ap()
    idxb = idxb_t.ap()
    taba = taba_t.ap()
    tabb = tabb_t.ap()
    out = out_t.ap()
    outv = out.rearrange("(p r) e -> p r e", p=128)

    with tile.TileContext(nc) as tc:
        with (
            tc.tile_pool(name="idxp", bufs=1) as ip,
            tc.tile_pool(name="work", bufs=BUFS) as wp,
        ):
            ia = ip.tile([128, cols], mybir.dt.int16, tag="ia")
            ib = ip.tile([128, cols], mybir.dt.int16, tag="ib")
            if CHUNK > 1:
                # chunked preload: first gathers only wait on their own chunk
                step = (cols + CHUNK - 1) // CHUNK
                for c0_ in range(0, cols, step):
                    c1_ = min(c0_ + step, cols)
                    nc.sync.dma_start(out=ia[:, c0_:c1_], in_=idxa[:, c0_:c1_])
                    nc.sync.dma_start(out=ib[:, c0_:c1_], in_=idxb[:, c0_:c1_])
            else:
                nc.sync.dma_start(out=ia[:], in_=idxa)
                nc.sync.dma_start(out=ib[:], in_=idxb)

            for t in range(ntiles):
                fb = FB if t < nfull else tail_rows
                ni = fb * 128
                c0 = (t * TT) // 16
                da = wp.tile([128, fb, 128], dt, tag="da")
                db = wp.tile([128, fb, 128], dt, tag="db")
                if NQUEUES > 1:
                    qa = (2 * t) % NQUEUES
                    qb = (2 * t + 1) % NQUEUES
                else:
                    qa = qb = 0
                nc.gpsimd.dma_gather(
                    da[:], taba, ia[:, c0 : c0 + ni // 16], ni, ni, D,
                    queue_num=qa,
                )
                nc.gpsimd.dma_gather(
                    db[:], tabb, ib[:, c0 : c0 + ni // 16], ni, ni, D,
                    queue_num=qb,
                )
                nc.vector.tensor_add(out=da[:], in0=da[:], in1=db[:])
                st = nc.scalar if (SSPLIT and t % 2) else nc.sync
                r0 = t * FB
                st.dma_start(out=outv[:, r0 : r0 + fb, :], in_=da[:])
    nc.compile()
    return nc


def _build_nc_peb():
    """A-table via dma_gather; B-table on the TensorEngine.

    Per tile: DVE builds two code-on-partition one-hots (is_equal of the
    host-prebroadcast cv stream against per-partition iota columns), PE
    contracts them against the SBUF-resident 256-row B table into PSUM
    ([tok, e]), DVE adds A-gather + PSUM, HWDGE stores. Removes the B
    gathers from both the SWDGE queues (Q7 descriptor-gen was the
    bottleneck) and the SDMA random-read load."""
    import concourse.bacc as bacc
    import concourse.mybir as mybir
    import concourse.tile as tile

    assert DTS == "f16"
    dt = mybir.dt.float16
    nfull = PER // TT
    tail_rows = RPC - nfull * FB
    ntiles = nfull + (1 if tail_rows else 0)
    cols = PADTOK // 16

    kw = {}
    if NQUEUES > 1:
        kw["num_swdge_queues"] = NQUEUES
    if SCRATCH:
        kw["dynamic_dma_scratch_size"] = SCRATCH
    nc = bacc.Bacc(
        "TRN2",
        target_bir_lowering=False,
        debug=False,
        enable_asserts=False,
        **kw,
    )
    nfull_ = PER // TT
    ntiles_ = nfull_ + 1
    tpp = -(-ntiles_ // 3)             # tiles per cvrow base partition (0/32/64)
    idxa_t = nc.dram_tensor("idxa", [128, cols], mybir.dt.int16, kind="ExternalInput")
    cdt = mybir.dt.uint8 if U8 else dt
    va = V * 2 if U8 else V * RA
    taba_t = nc.dram_tensor("taba", [va, D], dt, kind="ExternalInput")
    tabbs_t = nc.dram_tensor("tabbs", [256, D], dt, kind="ExternalInput")
    if BCMM:
        cvr_t = nc.dram_tensor("cvrow", [65, tpp * TT], dt, kind="ExternalInput")
        ones_t = nc.dram_tensor("ones1", [65, 128], dt, kind="ExternalInput")
    else:
        cvbc_t = nc.dram_tensor("cvbc", [128, PADTOK], cdt, kind="ExternalInput")
    iota_t = nc.dram_tensor("iota2", [128, 2, TT], cdt, kind="ExternalInput")
    out_t = nc.dram_tensor("out", [PADTOK, D], dt, kind="ExternalOutput")

    idxa = idxa_t.ap()
    taba = taba_t.ap()
    tabbs = tabbs_t.ap()
    if BCMM:
        cvr = cvr_t.ap()
        ones1 = ones_t.ap()
    else:
        cvbc = cvbc_t.ap()
    iota2 = iota_t.ap()
    out = out_t.ap()
    outv = out.rearrange("(p r) e -> p r e", p=128)

    with tile.TileContext(nc) as tc:
        with (
            tc.tile_pool(name="idxp", bufs=1) as ip,
            tc.tile_pool(name="work", bufs=BUFS) as wp,
            tc.tile_pool(name="psum", bufs=2 if BCMM else 4, space="PSUM") as pp,
            tc.tile_pool(name="psbc", bufs=2, space="PSUM") as pb,
        ):
            ia = ip.tile([128, cols], mybir.dt.int16, tag="ia")
            tbl = ip.tile([128, 2, 128], dt, tag="tb")   # [code, chunk, e]
            io2 = ip.tile([128, 2, TT], cdt, tag="io")
            nc.sync.dma_start(out=tbl[:, 0, :], in_=tabbs[0:128, :])
            nc.sync.dma_start(out=tbl[:, 1, :], in_=tabbs[128:256, :])
            nc.sync.dma_start(out=io2[:], in_=iota2)
            if BCMM:
                cvrs = ip.tile([65, tpp * TT], dt, tag="cr")
                on1 = ip.tile([65, 128], dt, tag="on")
                nc.sync.dma_start(out=cvrs[:], in_=cvr)
                nc.sync.dma_start(out=on1[:], in_=ones1)
            if CHUNK > 1:
                step = (cols + CHUNK - 1) // CHUNK
                for c0_ in range(0, cols, step):
                    c1_ = min(c0_ + step, cols)
                    nc.sync.dma_start(out=ia[:, c0_:c1_], in_=idxa[:, c0_:c1_])
            else:
                nc.sync.dma_start(out=ia[:], in_=idxa)

            for t in range(ntiles):
                fb = FB if t < nfull else tail_rows
                ni = fb * 128
                c0 = (t * TT) // 16
                da = wp.tile([128, fb, 128], dt, tag="da")
                cvt = wp.tile([128, fb * 128], cdt, tag="cv")
                oh = wp.tile([128, 2, fb, 128], dt, tag="oh")
                ps = pp.tile([128, fb * 128], mybir.dt.float32, tag="ps")
                psv = ps[:].rearrange("p (b e) -> p b e", e=128)
                if SPLITG and fb >= 2:
                    h = fb // 2
                    nh = h * 128
                    nc.gpsimd.dma_gather(
                        da[:, :h, :], taba, ia[:, c0 : c0 + nh // 16],
                        nh, nh, D, queue_num=(2 * t) % NQUEUES,
                    )
                    nc.gpsimd.dma_gather(
                        da[:, h:, :], taba,
                        ia[:, c0 + nh // 16 : c0 + ni // 16],
                        ni - nh, ni - nh, D, queue_num=(2 * t + 1) % NQUEUES,
                    )
                else:
                    qa = t % NQUEUES if NQUEUES > 1 else 0
                    nc.gpsimd.dma_gather(
                        da[:], taba, ia[:, c0 : c0 + ni // 16], ni, ni, D,
                        queue_num=qa,
                    )
                if BCMM:
                    pbc = pb.tile([128, fb * 128], mybir.dt.float32, tag="pb")
                    base = 32 * (t // tpp)             # 0, 32, or 64
                    j_ = t % tpp
                    for h0 in range(0, ni, 512):
                        h1 = min(h0 + 512, ni)
                        nc.tensor.matmul(
                            pbc[:, h0:h1],
                            on1[base : base + 1, :],
                            cvrs[base : base + 1, j_ * TT + h0 : j_ * TT + h1],
                            start=True,
                            stop=True,
                        )
                    nc.scalar.activation(
                        out=cvt[:],
                        in_=pbc[:],
                        func=mybir.ActivationFunctionType.Copy,
                    )
                else:
                    nc.scalar.dma_start(
                        out=cvt[:], in_=cvbc[:, t * TT : t * TT + ni]
                    )
                for ch in range(2):
                    ohv = oh[:, ch, :, :].rearrange("p b e -> p (b e)")
                    nc.vector.tensor_tensor(
                        out=ohv,
                        in0=cvt[:],
                        in1=io2[:, ch, : fb * 128],
                        op=mybir.AluOpType.is_equal,
                    )
                for b in range(fb):
                    nc.tensor.matmul(
                        psv[:, b, :],
                        oh[:, 0, b, :],
                        tbl[:, 0, :],
                        start=True,
                        stop=False,
                    )
                    nc.tensor.matmul(
                        psv[:, b, :],
                        oh[:, 1, b, :],
                        tbl[:, 1, :],
                        start=False,
                        stop=True,
                    )
                bb = wp.tile([128, fb, 128], dt, tag="bb")
                nc.scalar.activation(
                    out=bb[:].rearrange("p b e -> p (b e)"),
                    in_=ps[:],
                    func=mybir.ActivationFunctionType.Copy,
                )
                nc.vector.tensor_add(out=da[:], in0=da[:], in1=bb[:])
                r0 = t * FB
                nc.sync.dma_start(out=outv[:, r0 : r0 + fb, :], in_=da[:])
    nc.compile()
    return nc


def _get_nc():
    if "nc" not in _cache:
        _cache["nc"] = _build_nc_peb() if PEB else _build_nc()
    return _cache["nc"]


def _wrap_idx(arr_i16):
    """[PADTOK] int16 -> [128, COLS] dma_gather wrapped layout: stream slot i
    lives at [i % 16, i // 16]; the 16-row block is replicated to fill 128
    partitions."""
    w16 = arr_i16.reshape(-1, 16).T  # [16, COLS]
    return np.ascontiguousarray(np.tile(w16, (8, 1)))  # [128, COLS]


def _stream_token_order():
    """token id carried by each gather-stream slot.

    Stream slot s = t*TT + b*128 + p (tile t, block b, partition p) maps to
    output row p*RPC + t*FB + b, so each partition's store per tile is FB
    contiguous rows. The tail tile has tail_rows blocks."""
    nfull = PER // TT
    tail_rows = RPC - nfull * FB
    t_ = np.arange(nfull)[:, None, None]
    b_ = np.arange(FB)[None, :, None]
    p_ = np.arange(128)[None, None, :]
    head = (p_ * RPC + t_ * FB + b_).reshape(-1)
    parts = [head]
    if tail_rows:
        tb_ = np.arange(tail_rows)[:, None]
        tp_ = np.arange(128)[None, :]
        parts.append((tp_ * RPC + nfull * FB + tb_).reshape(-1))
    return np.concatenate(parts)


def kernel(
    fixed_features,
    idx0, val0, idx1, val1, idx2, val2, idx3, val3,
    fixed_table, tab0, tab1, tab2, tab3, W_fixed, W_sparse, b,
):
    from concourse.bass_utils import run_bass_kernel_spmd

    npdt = _npdt()
    ff = np.asarray(fixed_features)
    # combined sparse code per token; 256 = untouched sentinel (zero row).
    cv = np.full(N, 256, dtype=np.int32)
    for k, (ii, vv) in enumerate(
        ((idx0, val0), (idx1, val1), (idx2, val2), (idx3, val3))
    ):
        cv[np.asarray(ii)] = k * 64 + np.asarray(vv).astype(np.int32)

    ft = np.asarray(fixed_table, dtype=np.float32)
    wf = np.asarray(W_fixed, dtype=np.float32)
    ws = np.asarray(W_sparse, dtype=np.float32)
    bb = np.asarray(b, dtype=np.float32)
    taba1 = (ft @ wf + bb).astype(npdt)
    taba = np.ascontiguousarray(np.tile(taba1, (RA, 1)))
    tabs = np.concatenate(
        [np.asarray(t, dtype=np.float32) for t in (tab0, tab1, tab2, tab3)], axis=0
    )
    tabb1 = np.concatenate([tabs @ ws, np.zeros((1, D), np.float32)], axis=0)
    tabb = np.ascontiguousarray(np.tile(tabb1.astype(npdt), (REP, 1)))

    tok = _stream_token_order()                      # [PADTOK]
    valid = tok < PER
    tokc = np.where(valid, tok, 0)
    sr = np.arange(PADTOK)
    boff = (sr % REP).astype(np.int32) * NSPARSE     # B replica per stream slot
    aoff = (sr % RA).astype(np.int32) * V if RA > 1 else 0
    if PEB:
        iocol = np.stack([np.arange(128), 128 + np.arange(128)], axis=1)
        iota2 = np.ascontiguousarray(
            np.broadcast_to(iocol[:, :, None], (128, 2, TT))
        ).astype(np.uint8 if U8 else npdt)
        tabbs = np.ascontiguousarray(tabb1[:256].astype(npdt))
        if U8:
            tabaf = ft @ wf + bb
            taba = np.ascontiguousarray(
                np.concatenate([tabaf, tabaf - tabb1[0]], axis=0).astype(npdt)
            )

    in_maps = []
    for c in range(NCORES):
        sl = slice(c * PER, (c + 1) * PER)
        ffc = np.asarray(ff[sl]).astype(np.int32)[tokc]
        cvc = cv[sl][tokc]
        if PEB and U8:
            fa = np.where(valid, ffc + V * (cvc == 256), 0).astype(np.int16)
        else:
            fa = np.where(valid, ffc + aoff, 0).astype(np.int16)
        if PEB:
            cvs8 = np.where(valid, cvc, 256)
            cvs = (
                (cvs8 & 255).astype(np.uint8)
                if U8
                else cvs8.astype(npdt)
            )
            m = {
                "idxa": _wrap_idx(fa),
                "taba": taba,
                "tabbs": tabbs,
                "iota2": iota2,
            }
            if BCMM:
                nfull_ = PER // TT
                ntiles_ = nfull_ + 1
                tpp = -(-ntiles_ // 3)
                cvrow = np.full((65, tpp * TT), 256, dtype=npdt)
                for t_ in range(ntiles_):
                    seg = cvs[t_ * TT : min((t_ + 1) * TT, cvs.size)]
                    base, j_ = 32 * (t_ // tpp), t_ % tpp
                    cvrow[base, j_ * TT : j_ * TT + seg.size] = seg
                m["cvrow"] = np.ascontiguousarray(cvrow)
                ones65 = np.zeros((65, 128), dtype=npdt)
                ones65[[0, 32, 64], :] = 1
                m["ones1"] = ones65
            else:
                m["cvbc"] = np.ascontiguousarray(
                    np.broadcast_to(cvs[None, :], (128, PADTOK))
                )
            in_maps.append(m)
            continue
        fbv = np.where(valid, cvc + boff, 256).astype(np.int16)
        in_maps.append(
            {
                "idxa": _wrap_idx(fa),
                "idxb": _wrap_idx(fbv),
                "taba": taba,
                "tabb": tabb,
            }
        )

    nc = _get_nc()
    res = run_bass_kernel_spmd(nc, in_maps, core_ids=list(range(NCORES)))
    _cache["last_results"] = res
    out = np.concatenate(
        [
            np.asarray(res.results[c]["out"][:PER], dtype=np.float32)
            for c in range(NCORES)
        ],
        axis=0,
    )
    return out
